# revision 21
# baseline (speedup 1.0000x reference)
"""MoE top-2 routing kernel for Trainium2 (8 NeuronCores, SPMD data-parallel).

Problem (fixed shapes): x [4, 2048, 1024] fp32, 8 experts of [1024, 1024],
top-2 routing by softmax(x @ w_router.T + b_router) (monotone -> top-2 of
logits), output = UNweighted sum of the selected experts' linear outputs.

Sharding: data-parallel over tokens. Each of the 8 cores takes 1024 tokens
(x pre-transposed on host to [d_in, tok] so all device matmuls contract over
the partition dim with contiguous DMA loads), with the expert weights
replicated (host pre-transposed to [d_in, d_out]).

Per-core device program (Tile framework):
  1. Router logits in fp32 via PE matmuls (+b_router folded in as a K=1
     rank-1 matmul with a ones row); top-2 membership mask computed with the
     DVE max8 instruction (mask = logits >= 2nd max).
  2. acc[m,n] initialized with sum_e mask_e * b_e via a rank-1 matmul
     (maskT [8,128] @ b_experts [8,512]); maskT produced by a PE transpose.
  3. For each expert: y_e = x @ w_e^T via fp32r matmuls (full-rate fp32),
     then acc += mask_e * psum with one fused DVE scalar_tensor_tensor op.
"""

import sys

if "/opt/trn_rl_repo" not in sys.path:
    sys.path.insert(0, "/opt/trn_rl_repo")

import numpy as np

import concourse.bass as bass
import concourse.mybir as mybir
from concourse.bass_utils import run_bass_kernel_spmd
from concourse.masks import make_identity
from concourse.tile import TileContext

P = 128
B, T, DIN, DOUT, E = 4, 2048, 1024, 1024, 8
NCORES = 8
TOK = B * T // NCORES  # 1024 tokens per core
KT = DIN // P          # 8 contraction tiles
MT = TOK // P          # 8 token tiles
NH = 512               # psum free-dim half
NT = DOUT // NH        # 2
F32 = mybir.dt.float32
F32R = mybir.dt.float32r
BF16 = mybir.dt.bfloat16

LAST_RESULTS = None  # BassKernelResults of the most recent run (for profiling)


def split_excess_waits(nc: bass.Bass, limit: int = 1) -> int:
    """Hoist excess per-instruction semaphore waits onto injected NoOps.

    The walrus build in this container rejects instructions carrying more
    than a couple of sync waits (CoreV3GenImpl setupSyncWait: "Too many sync
    wait commands") — the Tile tail drain routinely exceeds that. Splitting
    the waits across preceding same-engine NoOps is semantically identical.
    """
    n_new = 0
    for bb in nc.main_func.blocks:
        newlist = []
        for ins in bb.instructions:
            si = ins.sync_info
            if si is not None and si.on_wait and len(si.on_wait) > limit:
                waits = list(si.on_wait)
                keep = waits[:limit]
                extra = waits[limit:]
                for i in range(0, len(extra), limit):
                    n_new += 1
                    nop = mybir.InstNoOp(
                        name=f"{ins.name}-waitsplit{i}",
                        engine=ins.engine,
                        ins=[],
                        outs=[],
                        sync_info=mybir.SyncInfo(
                            on_wait=extra[i : i + limit], on_update=[]
                        ),
                    )
                    newlist.append(nop)
                si.on_wait = keep
            newlist.append(ins)
        bb.instructions[:] = newlist
    return n_new


def build_dense() -> bass.Bass:
    nc = bass.Bass()
    # Router input stays fp32 (top-2 ranking margins are ~1e-4); expert
    # matmuls run in bf16 (full-rate with fast weight loads, half the DMA).
    xT = nc.dram_tensor("xT", [DIN, TOK], F32, kind="ExternalInput")
    xTb = nc.dram_tensor("xTb", [DIN, TOK], BF16, kind="ExternalInput")
    wT = nc.dram_tensor("wT", [E * DIN, DOUT], BF16, kind="ExternalInput")
    wrk = nc.dram_tensor("wrk", [P, KT * E], F32, kind="ExternalInput")
    brx = nc.dram_tensor("brx", [1, E], F32, kind="ExternalInput")
    be = nc.dram_tensor("be", [E, DOUT], F32, kind="ExternalInput")
    y = nc.dram_tensor("y", [TOK, DOUT], F32, kind="ExternalOutput")

    with TileContext(nc) as tc:
        with (
            tc.tile_pool(name="const", bufs=1) as cpool,
            tc.tile_pool(name="xp", bufs=1) as xpool,
            tc.tile_pool(name="wp", bufs=2) as wpool,
            tc.tile_pool(name="accp", bufs=1) as accpool,
            tc.tile_pool(name="mp", bufs=1) as mpool,
            tc.tile_pool(name="sp", bufs=2) as spool,
            tc.tile_pool(name="psY", bufs=4, space="PSUM") as psY,
            tc.tile_pool(name="psS", bufs=1, space="PSUM") as psS,
        ):
            ident = cpool.tile([P, P], F32, tag="ident")
            make_identity(nc, ident)
            ones = cpool.tile([1, P], F32, tag="ones")
            nc.vector.memset(ones, 1.0)

            wr_sb = cpool.tile([P, KT * E], F32, tag="wr")
            nc.sync.dma_start(out=wr_sb, in_=wrk[:, :])
            br_sb = cpool.tile([1, E], F32, tag="br")
            nc.sync.dma_start(out=br_sb, in_=brx[:, :])
            be_sb = cpool.tile([E, DOUT], F32, tag="be")
            nc.sync.dma_start(out=be_sb, in_=be[:, :])

            xts, xtbs = [], []
            for k in range(KT):
                xt = xpool.tile([P, TOK], F32, tag=f"xt{k}")
                nc.sync.dma_start(out=xt, in_=xT[k * P : (k + 1) * P, :])
                xts.append(xt)
                xtb = xpool.tile([P, TOK], BF16, tag=f"xtb{k}")
                nc.sync.dma_start(out=xtb, in_=xTb[k * P : (k + 1) * P, :])
                xtbs.append(xtb)

            # --- Router: logits -> top-2 mask (and its transpose) per token tile
            masks, maskTs = [], []
            for m in range(MT):
                psl = psS.tile([P, E], F32, tag="psl")
                for k in range(KT):
                    nc.tensor.matmul(
                        psl,
                        lhsT=xts[k][:, m * P : (m + 1) * P],
                        rhs=wr_sb[:, k * E : (k + 1) * E],
                        start=(k == 0),
                        stop=False,
                    )
                nc.tensor.matmul(psl, lhsT=ones, rhs=br_sb, start=False, stop=True)
                lsb = spool.tile([P, E], F32, tag="lsb")
                nc.vector.tensor_copy(lsb, psl)
                mx = spool.tile([P, E], F32, tag="mx")
                nc.vector.max(mx, lsb)
                msk = mpool.tile([P, E], F32, tag=f"msk{m}")
                nc.vector.tensor_tensor(
                    out=msk,
                    in0=lsb,
                    in1=mx[:, 1:2].to_broadcast([P, E]),
                    op=mybir.AluOpType.is_ge,
                )
                pst = psS.tile([E, P], F32, tag="pst")
                nc.tensor.transpose(pst, msk, ident)
                mT = mpool.tile([E, P], F32, tag=f"mT{m}")
                nc.vector.tensor_copy(mT, pst)
                masks.append(msk)
                maskTs.append(mT)

            # --- acc init = sum_e mask_e * b_e  (rank-8 matmul)
            accs = {}
            for m in range(MT):
                for n in range(NT):
                    psb = psS.tile([P, NH], F32, tag="psb")
                    nc.tensor.matmul(
                        psb,
                        lhsT=maskTs[m],
                        rhs=be_sb[:, n * NH : (n + 1) * NH],
                        start=True,
                        stop=True,
                    )
                    acc = accpool.tile([P, NH], F32, tag=f"acc{m}_{n}")
                    nc.vector.tensor_copy(acc, psb)
                    accs[(m, n)] = acc

            # --- Experts: dense matmul + fused masked accumulate
            for e in range(E):
                wts = []
                for k in range(KT):
                    wt = wpool.tile([P, DOUT], BF16, tag=f"w{k}")
                    nc.sync.dma_start(
                        out=wt, in_=wT[(e * KT + k) * P : (e * KT + k + 1) * P, :]
                    )
                    wts.append(wt)
                for m in range(MT):
                    for n in range(NT):
                        ps = psY.tile([P, NH], F32, tag="ps")
                        for k in range(KT):
                            nc.tensor.matmul(
                                ps,
                                lhsT=xtbs[k][:, m * P : (m + 1) * P],
                                rhs=wts[k][:, n * NH : (n + 1) * NH],
                                start=(k == 0),
                                stop=(k == KT - 1),
                            )
                        nc.vector.scalar_tensor_tensor(
                            out=accs[(m, n)],
                            in0=ps,
                            scalar=masks[m][:, e : e + 1],
                            in1=accs[(m, n)],
                            op0=mybir.AluOpType.mult,
                            op1=mybir.AluOpType.add,
                        )

            for m in range(MT):
                for n in range(NT):
                    nc.sync.dma_start(
                        out=y[m * P : (m + 1) * P, n * NH : (n + 1) * NH],
                        in_=accs[(m, n)],
                    )
    return nc


def kernel(x, w_router, b_router, w_experts, b_experts):
    global LAST_RESULTS
    import ml_dtypes

    x = np.ascontiguousarray(x, np.float32)
    xf = x.reshape(B * T, DIN)
    wT_full = np.ascontiguousarray(
        np.asarray(w_experts, np.float32).transpose(0, 2, 1).reshape(E * DIN, DOUT)
    ).astype(ml_dtypes.bfloat16)
    # [P, KT*E]: column block k holds w_router.T rows [k*P, (k+1)*P)
    wrk = np.ascontiguousarray(
        np.asarray(w_router, np.float32).T.reshape(KT, P, E).transpose(1, 0, 2).reshape(P, KT * E)
    )
    brx = np.ascontiguousarray(np.asarray(b_router, np.float32).reshape(1, E))
    be = np.ascontiguousarray(np.asarray(b_experts, np.float32))

    nc = build_dense()
    split_excess_waits(nc)
    in_maps = []
    for c in range(NCORES):
        xTc = np.ascontiguousarray(xf[c * TOK : (c + 1) * TOK].T)
        in_maps.append(
            {
                "xT": xTc,
                "xTb": xTc.astype(ml_dtypes.bfloat16),
                "wT": wT_full,
                "wrk": wrk,
                "brx": brx,
                "be": be,
            }
        )
    res = run_bass_kernel_spmd(nc, in_maps, list(range(NCORES)))
    LAST_RESULTS = res
    yout = np.concatenate([res.results[c]["y"] for c in range(NCORES)], axis=0)
    return yout.reshape(B, T, DOUT)
